# revision 46
# baseline (speedup 1.0000x reference)
"""FMoE forward (NaiveGate top-2, per-expert FFN, score-weighted combine) on 8 trn2 cores.

Strategy: hybrid expert-parallel x tensor-parallel. Cores split into 2
groups of 4; each group owns 4 experts; within a group each core holds a
1024-wide hidden slice of its 4 experts' W1/W2 (16MB resident in SBUF) and
processes ALL of the group's dispatched token-expert pairs against its
slice. Per-core DMA is ~33MB (vs ~49MB for pure 8-way TP), which keeps
the HBM stream comfortably under the per-core budget, and load balance is
near-perfect: the SPMD slab plan uses the elementwise max of the two
groups' (sorted) segment lengths, with the expert->group partition chosen
to minimize that padding (~1% over the ideal 4096 pairs/group).

Device kernel (per core, fp16 matmuls, fp32 accum):
  mm1: stationary = W1 slice chunk [128k, 128h], moving = X^T slab
       [128k, S] -> H^T chunk [128h, S] PSUM (8 k-chunks); ScalarE
       tanh-gelu (+b1), VectorE multiply by per-pair gate weight (fp16).
  mm2: stationary = W2 slice chunk [128h, 128d], moving = H^T chunk
       -> Y^T [128d, S] PSUM (8 h-chunks), fp16 copy out (split across
       scalar+vector), one slab-blocked DMA out.
Software pipeline: mm2 of slab s is emitted after mm1 of slab s+1 so the
PE never stalls on a slab's last gelu. DMA issue is spread over engine
sequencers (gpsimd: weight stream + out, sync: X, scalar: b1 + gate rows)
and the 16MB weight stream is popped in ~1MB pieces per slab boundary in
strict first-need order.
"""

import os
import sys

import numpy as np

for _p in ("/opt/trn_rl_repo",):
    if _p not in sys.path and os.path.isdir(_p):
        sys.path.insert(0, _p)

N_TOKENS = 4096
D_MODEL = 1024
D_HIDDEN = 4096
N_EXPERT = 8
TOP_K = 2
P = 128
KO = D_MODEL // P  # 8 contraction chunks for mm1
NCORES = 8
NGROUPS = 2
TPK = NCORES // NGROUPS  # 4-way tensor parallel within a group
EPG = N_EXPERT // NGROUPS  # 4 experts per group
HB = D_HIDDEN // TPK  # 1024-wide hidden slice per core
HO = HB // P  # 8 h-chunks per core for mm2 contraction
DM = D_MODEL // P  # 8 output-partition chunks of Y^T
SLAB = 512  # max moving-dim per matmul (hard ISA limit, one PSUM bank)
WARMUP_MM = 12

# filled by kernel() for test harness introspection
last_results = None

_nc_cache = {}


def _even_split(L, cap=SLAB):
    """Split L (a multiple of 4) into even parts <= cap, each a multiple of 4."""
    q = L // 4
    n = -(-L // cap)
    base, extra = divmod(q, n)
    return [4 * (base + 1)] * extra + [4 * base] * (n - extra)


def _make_slabs(plan):
    """Slab plan from padded segment lengths: list of (seg_idx, S, col0).
    Big even slabs only -- short matmuls pay exposed LDWEIGHTS (~60-107ns
    per MM), so no small lead-in/taper slabs."""
    slabs = []
    col0 = 0
    for i, Lp in enumerate(plan):
        if Lp == 0:
            continue
        for S in _even_split(Lp):
            slabs.append((i, S, col0))
            col0 += S
    return tuple(slabs), col0


def _group_split(loads):
    """Choose the 4+4 expert partition minimizing the shared (pairwise-max)
    padded plan, and return (groups, plan) with groups' experts sorted by
    descending load."""
    import itertools

    ids = list(range(N_EXPERT))
    best = None
    for combo in itertools.combinations(ids[1:], EPG - 1):
        ga = (0,) + combo
        gb = tuple(i for i in ids if i not in ga)
        la = sorted((loads[e] for e in ga), reverse=True)
        lb = sorted((loads[e] for e in gb), reverse=True)
        plan = tuple(-(-max(a, b) // 4) * 4 for a, b in zip(la, lb))
        cost = sum(plan)
        if best is None or cost < best[0]:
            ea = tuple(sorted(ga, key=lambda e: -loads[e]))
            eb = tuple(sorted(gb, key=lambda e: -loads[e]))
            best = (cost, (ea, eb), plan)
    (ea, eb), plan = best[1], best[2]
    # put the segment whose even-split slabs are largest first: slower
    # ho-group consumption in mm1(0) gives the startup weight stream the
    # most margin (the DMA lead-in race is the noisiest part of the run)
    order = sorted(range(len(plan)), key=lambda i: -min(_even_split(plan[i])))
    ea = tuple(ea[i] for i in order)
    eb = tuple(eb[i] for i in order)
    plan = tuple(plan[i] for i in order)
    return (ea, eb), plan


def _build_bass(slabs, cols):
    import concourse.mybir as mybir
    from concourse import bacc
    from concourse.tile import TileContext

    f16 = mybir.dt.float16
    f32 = mybir.dt.float32
    GELU = mybir.ActivationFunctionType.Gelu_apprx_tanh

    nc = bacc.Bacc("TRN2", target_bir_lowering=False, debug=False, num_devices=NCORES)

    SMAX = max(S for _, S, _ in slabs)

    x_d = nc.declare_dram_parameter("x", [P, KO * cols], f16, isOutput=False)
    w1_d = nc.declare_dram_parameter("w1", [EPG, P, KO, HB], f16, isOutput=False)
    w2_d = nc.declare_dram_parameter("w2", [EPG, P, HO, D_MODEL], f16, isOutput=False)
    b1_d = nc.declare_dram_parameter("b1", [P, EPG, HO], f32, isOutput=False)
    wb_d = nc.declare_dram_parameter("wb", [P, cols], f16, isOutput=False)
    # slab-blocked output: per slab a contiguous [P, DM*S] block at DM*col0
    out_d = nc.declare_dram_parameter("out", [P, DM * cols], f16, isOutput=True)

    seg_first_slab = {}
    for si, (sg, S, c0) in enumerate(slabs):
        seg_first_slab.setdefault(sg, si)
    segs_used = sorted(seg_first_slab)

    with TileContext(nc) as tc:
        with (
            tc.tile_pool(name="wpool", bufs=1) as wpool,
            tc.tile_pool(name="xpool", bufs=3) as xpool,
            tc.tile_pool(name="wbpool", bufs=3) as wbpool,
            tc.tile_pool(name="hpool", bufs=2) as hpool,
            tc.tile_pool(name="ypool", bufs=2) as ypool,
            tc.tile_pool(name="ps1", bufs=3, space="PSUM") as ps1,
            tc.tile_pool(name="ps2", bufs=4, space="PSUM") as ps2,
        ):
            # PE warm-up: dependency-free matmuls keep PE busy (and HAM
            # warming) through the preamble barrier + DMA lead-in.
            warm = wpool.tile([P, 512], f16)
            nc.vector.memset(warm, 0.0)
            wps = ps1.tile([P, SMAX], mybir.dt.float32, tag="hps")
            for _ in range(WARMUP_MM):
                nc.tensor.matmul(
                    wps[:, :512], lhsT=warm[:, :P], rhs=warm, start=True, stop=True
                )

            b1_sb = wpool.tile([P, EPG, HO], f32)
            w1_sb = wpool.tile([P, EPG, KO, HB], f16)
            w2_sb = wpool.tile([P, EPG, HO, D_MODEL], f16)

            # Weight stream in strict first-need order, ~1MB pieces so the
            # early HBM window (8 cores all loading) isn't oversubscribed.
            wq = []
            s0 = segs_used[0]
            for sg in segs_used:
                if sg == s0:
                    wq.append(("w1", sg, 0, 128))
                    wq.append(("w1", sg, 128, 512))
                    wq.append(("w1", sg, 512, HB))
                else:
                    wq.append(("w1", sg, 0, 512))
                    wq.append(("w1", sg, 512, HB))
                wq.append(("w2", sg, 0, 512))
                wq.append(("w2", sg, 512, D_MODEL))

            def issue_weight():
                if not wq:
                    return
                kind, sg, lo, hi = wq.pop(0)
                if kind == "w1":
                    nc.gpsimd.dma_start(
                        w1_sb[:, sg, :, lo:hi], w1_d[sg][:, :, lo:hi]
                    )
                else:
                    nc.gpsimd.dma_start(
                        w2_sb[:, sg, :, lo:hi], w2_d[sg][:, :, lo:hi]
                    )

            nc.scalar.dma_start(b1_sb, b1_d[:, :, :])
            # upfront: all of segment 0's weights (~4MB); the rest of the
            # stream is paced behind the per-slab out-DMAs on gpsimd's
            # in-order queue, so the 16MB never floods the early HBM window
            for _ in range(5):
                issue_weight()

            def mm1_slab(si):
                sg, S, c0 = slabs[si]
                x_sb = xpool.tile([P, KO, SMAX], f16, tag="x", name="x_sb")[:, :, :S]
                x_src = x_d[:, KO * c0 : KO * (c0 + S)].rearrange(
                    "p (ko t) -> p ko t", t=S
                )
                # two dma_starts -> two parallel HW queues
                nc.sync.dma_start(x_sb[:, : KO // 2, :], x_src[:, : KO // 2, :])
                nc.sync.dma_start(x_sb[:, KO // 2 :, :], x_src[:, KO // 2 :, :])
                wb_t = wbpool.tile([P, SMAX], f16, tag="wb", name="wb_t")[:, :S]
                nc.scalar.dma_start(wb_t, wb_d[:, c0 : c0 + S])
                h_sb = hpool.tile([P, HO, SMAX], f16, tag="h", name="h_sb")[:, :, :S]
                for ho in range(HO):
                    hps = ps1.tile(
                        [P, SMAX], mybir.dt.float32, tag="hps", name="hps"
                    )[:, :S]
                    for k in range(KO):
                        nc.tensor.matmul(
                            hps,
                            lhsT=w1_sb[:, sg, k, ho * P : (ho + 1) * P],
                            rhs=x_sb[:, k, :],
                            start=(k == 0),
                            stop=(k == KO - 1),
                        )
                    nc.scalar.activation(
                        h_sb[:, ho, :], hps, GELU, bias=b1_sb[:, sg, ho : ho + 1]
                    )
                    # fold the per-pair gate weight into H (fp16, free dim)
                    nc.vector.tensor_mul(h_sb[:, ho, :], h_sb[:, ho, :], wb_t)
                return h_sb

            def mm2_slab(si, h_sb):
                sg, S, c0 = slabs[si]
                last2 = si >= len(slabs) - 2
                y_all = ypool.tile([P, DM, SMAX], f16, tag="y", name="y_sb")[:, :, :S]
                out_dst = out_d[:, DM * c0 : DM * (c0 + S)].rearrange(
                    "p (m t) -> p m t", t=S
                )
                for m in range(DM):
                    yps = ps2.tile(
                        [P, SMAX], mybir.dt.float32, tag="yps", name="yps"
                    )[:, :S]
                    for ho in range(HO):
                        nc.tensor.matmul(
                            yps,
                            lhsT=w2_sb[:, sg, ho, m * P : (m + 1) * P],
                            rhs=h_sb[:, ho, :],
                            start=(ho == 0),
                            stop=(ho == HO - 1),
                        )
                    # PSUM->SBUF fp16 copy split across both engines; on
                    # the final slab use quarter strips so the last copy's
                    # latency (which is pure tail) is minimal
                    if si == len(slabs) - 1:
                        q = (S // 16) * 4
                        nc.scalar.copy(y_all[:, m, :q], yps[:, :q])
                        nc.vector.tensor_copy(y_all[:, m, q : 2 * q], yps[:, q : 2 * q])
                        nc.scalar.copy(y_all[:, m, 2 * q : 3 * q], yps[:, 2 * q : 3 * q])
                        nc.vector.tensor_copy(y_all[:, m, 3 * q :], yps[:, 3 * q :])
                    else:
                        h2 = (S // 8) * 4
                        nc.scalar.copy(y_all[:, m, :h2], yps[:, :h2])
                        nc.vector.tensor_copy(y_all[:, m, h2:], yps[:, h2:])
                    # tail slabs: flush per-m / m-pairs from the (idle by
                    # then) sync engine so the out-DMA overlaps the matmuls
                    if si == len(slabs) - 1:
                        nc.sync.dma_start(out_dst[:, m : m + 1], y_all[:, m : m + 1])
                    elif last2 and m % 2 == 1:
                        nc.sync.dma_start(
                            out_dst[:, m - 1 : m + 1], y_all[:, m - 1 : m + 1]
                        )
                if not last2:
                    nc.gpsimd.dma_start(out_dst, y_all)
                    # pace the weight stream behind this slab's out-DMA
                    issue_weight()
                    issue_weight()

            # software pipeline: mm1(s+1) before mm2(s)
            h_prev = mm1_slab(0)
            for si in range(1, len(slabs)):
                h_cur = mm1_slab(si)
                mm2_slab(si - 1, h_prev)
                h_prev = h_cur
            mm2_slab(len(slabs) - 1, h_prev)
    nc.compile()
    return nc


def _route(moe_inp, Wg, bg):
    """Host gate: replicates NaiveGate (linear logits, top-2, softmax over the
    selected logits). Returns per-expert (token_idx, combine_weight)."""
    logits = moe_inp.astype(np.float32) @ Wg.astype(np.float32) + bg.astype(np.float32)
    order = np.argsort(-logits, axis=1, kind="stable")  # ties -> lower index first
    top_idx = order[:, :TOP_K]
    top_val = np.take_along_axis(logits, top_idx, axis=1)
    m = top_val.max(axis=1, keepdims=True)
    e = np.exp(top_val - m)
    gate = (e / e.sum(axis=1, keepdims=True)).astype(np.float32)
    toks, weights = [], []
    for ex in range(N_EXPERT):
        mask = top_idx == ex  # [N, K]; each token matches at most one slot
        t = np.nonzero(mask.any(axis=1))[0]
        w = gate[mask]  # row-major -> ascending token order, matches t
        toks.append(t)
        weights.append(w)
    return toks, weights


def kernel(**inputs):
    global last_results
    from concourse.bass_utils import run_bass_kernel_spmd

    moe_inp = np.asarray(inputs["moe_inp"], dtype=np.float32)
    Wg = np.asarray(inputs["Wg"], dtype=np.float32)
    bg = np.asarray(inputs["bg"], dtype=np.float32)
    W1 = np.asarray(inputs["W1"], dtype=np.float32)
    b1 = np.asarray(inputs["b1"], dtype=np.float32)
    W2 = np.asarray(inputs["W2"], dtype=np.float32)
    b2 = np.asarray(inputs["b2"], dtype=np.float32)

    toks, weights = _route(moe_inp, Wg, bg)
    loads = [len(t) for t in toks]
    groups, plan = _group_split(loads)
    slabs, cols = _make_slabs(plan)

    if slabs not in _nc_cache:
        _nc_cache[slabs] = _build_bass(slabs, cols)
    nc = _nc_cache[slabs]

    seg_c0 = {}
    for sg, S, c0 in slabs:
        if sg not in seg_c0:
            seg_c0[sg] = c0

    # per-group dispatched X^T / gate rows (segments padded to the plan)
    garrs = []
    for g in range(NGROUPS):
        xT = np.zeros((D_MODEL, cols), dtype=np.float16)
        wrow = np.zeros((cols,), dtype=np.float16)
        for i, e in enumerate(groups[g]):
            c0, L = seg_c0[i], loads[e]
            xT[:, c0 : c0 + L] = moe_inp[toks[e]].T
            wrow[c0 : c0 + L] = weights[e]
        blocks = []
        for sg, S, c0 in slabs:
            blocks.append(
                xT[:, c0 : c0 + S].reshape(KO, P, S).transpose(1, 0, 2).reshape(P, KO * S)
            )
        x_arr = np.ascontiguousarray(np.concatenate(blocks, axis=1))
        wb_arr = np.ascontiguousarray(np.broadcast_to(wrow, (P, cols)))
        garrs.append((x_arr, wb_arr))

    in_maps = []
    for c in range(NCORES):
        g, s = divmod(c, TPK)
        gex = list(groups[g])
        lo, hi = s * HB, (s + 1) * HB
        w1_arr = np.ascontiguousarray(
            W1[gex][:, :, lo:hi]
            .astype(np.float16)
            .reshape(EPG, KO, P, HB)
            .transpose(0, 2, 1, 3)
        )
        w2_arr = np.ascontiguousarray(
            W2[gex][:, lo:hi, :]
            .astype(np.float16)
            .reshape(EPG, HO, P, D_MODEL)
            .transpose(0, 2, 1, 3)
        )
        b1_arr = np.ascontiguousarray(
            b1[gex][:, lo:hi].reshape(EPG, HO, P).transpose(2, 0, 1)
        )
        in_maps.append(
            {
                "x": garrs[g][0],
                "w1": w1_arr,
                "w2": w2_arr,
                "b1": b1_arr,
                "wb": garrs[g][1],
            }
        )

    last_results = run_bass_kernel_spmd(nc, in_maps, core_ids=list(range(NCORES)))

    # host combine: per group sum the 4 hidden-slice partials, decode the
    # slab-blocked layout, scatter by segment
    out = np.zeros((N_TOKENS, D_MODEL), dtype=np.float32)
    for g in range(NGROUPS):
        raw = np.zeros((P, DM * cols), dtype=np.float32)
        for s in range(TPK):
            raw += last_results.results[g * TPK + s]["out"].astype(np.float32)
        yT = np.empty((D_MODEL, cols), dtype=np.float32)
        for sg, S, c0 in slabs:
            blk = raw[:, DM * c0 : DM * (c0 + S)].reshape(P, DM, S)
            yT[:, c0 : c0 + S] = blk.transpose(1, 0, 2).reshape(D_MODEL, S)
        for i, e in enumerate(groups[g]):
            c0, L = seg_c0[i], loads[e]
            out[toks[e]] += yT[:, c0 : c0 + L].T + weights[e][:, None] * b2[e][None, :]
    return out


if __name__ == "__main__":
    rng = np.random.default_rng(0)
    demo = {
        "moe_inp": rng.standard_normal((N_TOKENS, D_MODEL), dtype=np.float32),
        "attn_weights": rng.random((4, N_TOKENS, N_TOKENS), dtype=np.float32),
        "Wg": rng.standard_normal((D_MODEL, N_EXPERT), dtype=np.float32) / 32,
        "bg": np.zeros((N_EXPERT,), np.float32),
        "W1": rng.standard_normal((N_EXPERT, D_MODEL, D_HIDDEN), dtype=np.float32) / 32,
        "b1": np.zeros((N_EXPERT, D_HIDDEN), np.float32),
        "W2": rng.standard_normal((N_EXPERT, D_HIDDEN, D_MODEL), dtype=np.float32) / 64,
        "b2": np.zeros((N_EXPERT, D_MODEL), np.float32),
    }
    o = kernel(**demo)
    print(o.shape, o.dtype)


# revision 47
# speedup vs baseline: 1.0108x; 1.0108x over previous
"""FMoE forward (NaiveGate top-2, per-expert FFN, score-weighted combine) on 8 trn2 cores.

Strategy: hybrid expert-parallel x tensor-parallel. Cores split into 2
groups of 4; each group owns 4 experts; within a group each core holds a
1024-wide hidden slice of its 4 experts' W1/W2 (16MB resident in SBUF) and
processes ALL of the group's dispatched token-expert pairs against its
slice. Per-core DMA is ~33MB (vs ~49MB for pure 8-way TP), which keeps
the HBM stream comfortably under the per-core budget, and load balance is
near-perfect: the SPMD slab plan uses the elementwise max of the two
groups' (sorted) segment lengths, with the expert->group partition chosen
to minimize that padding (~1% over the ideal 4096 pairs/group).

Device kernel (per core, fp16 matmuls, fp32 accum):
  mm1: stationary = W1 slice chunk [128k, 128h], moving = X^T slab
       [128k, S] -> H^T chunk [128h, S] PSUM (8 k-chunks); ScalarE
       tanh-gelu (+b1), VectorE multiply by per-pair gate weight (fp16).
  mm2: stationary = W2 slice chunk [128h, 128d], moving = H^T chunk
       -> Y^T [128d, S] PSUM (8 h-chunks), fp16 copy out (split across
       scalar+vector), one slab-blocked DMA out.
Software pipeline: mm2 of slab s is emitted after mm1 of slab s+1 so the
PE never stalls on a slab's last gelu. DMA issue is spread over engine
sequencers (gpsimd: weight stream + out, sync: X, scalar: b1 + gate rows)
and the 16MB weight stream is popped in ~1MB pieces per slab boundary in
strict first-need order.
"""

import os
import sys

import numpy as np

for _p in ("/opt/trn_rl_repo",):
    if _p not in sys.path and os.path.isdir(_p):
        sys.path.insert(0, _p)

N_TOKENS = 4096
D_MODEL = 1024
D_HIDDEN = 4096
N_EXPERT = 8
TOP_K = 2
P = 128
KO = D_MODEL // P  # 8 contraction chunks for mm1
NCORES = 8
NGROUPS = 2
TPK = NCORES // NGROUPS  # 4-way tensor parallel within a group
EPG = N_EXPERT // NGROUPS  # 4 experts per group
HB = D_HIDDEN // TPK  # 1024-wide hidden slice per core
HO = HB // P  # 8 h-chunks per core for mm2 contraction
DM = D_MODEL // P  # 8 output-partition chunks of Y^T
SLAB = 512  # max moving-dim per matmul (hard ISA limit, one PSUM bank)
WARMUP_MM = 12

# filled by kernel() for test harness introspection
last_results = None

_nc_cache = {}


def _even_split(L, cap=SLAB):
    """Split L (a multiple of 4) into even parts <= cap, each a multiple of 4."""
    q = L // 4
    n = -(-L // cap)
    base, extra = divmod(q, n)
    return [4 * (base + 1)] * extra + [4 * base] * (n - extra)


def _make_slabs(plan):
    """Slab plan from padded segment lengths: list of (seg_idx, S, col0).
    Big even slabs only -- short matmuls pay exposed LDWEIGHTS (~60-107ns
    per MM), so no small lead-in/taper slabs."""
    slabs = []
    col0 = 0
    for i, Lp in enumerate(plan):
        if Lp == 0:
            continue
        for S in _even_split(Lp):
            slabs.append((i, S, col0))
            col0 += S
    return tuple(slabs), col0


def _group_split(loads):
    """Choose the 4+4 expert partition minimizing the shared (pairwise-max)
    padded plan, and return (groups, plan) with groups' experts sorted by
    descending load."""
    import itertools

    ids = list(range(N_EXPERT))
    best = None
    for combo in itertools.combinations(ids[1:], EPG - 1):
        ga = (0,) + combo
        gb = tuple(i for i in ids if i not in ga)
        la = sorted((loads[e] for e in ga), reverse=True)
        lb = sorted((loads[e] for e in gb), reverse=True)
        plan = tuple(-(-max(a, b) // 4) * 4 for a, b in zip(la, lb))
        cost = sum(plan)
        if best is None or cost < best[0]:
            ea = tuple(sorted(ga, key=lambda e: -loads[e]))
            eb = tuple(sorted(gb, key=lambda e: -loads[e]))
            best = (cost, (ea, eb), plan)
    (ea, eb), plan = best[1], best[2]
    # put the segment whose even-split slabs are largest first: slower
    # ho-group consumption in mm1(0) gives the startup weight stream the
    # most margin (the DMA lead-in race is the noisiest part of the run)
    order = sorted(range(len(plan)), key=lambda i: -min(_even_split(plan[i])))
    ea = tuple(ea[i] for i in order)
    eb = tuple(eb[i] for i in order)
    plan = tuple(plan[i] for i in order)
    return (ea, eb), plan


def _build_bass(slabs, cols):
    import concourse.mybir as mybir
    from concourse import bacc
    from concourse.tile import TileContext

    f16 = mybir.dt.float16
    f32 = mybir.dt.float32
    GELU = mybir.ActivationFunctionType.Gelu_apprx_tanh

    nc = bacc.Bacc("TRN2", target_bir_lowering=False, debug=False, num_devices=NCORES)

    SMAX = max(S for _, S, _ in slabs)

    x_d = nc.declare_dram_parameter("x", [P, KO * cols], f16, isOutput=False)
    w1_d = nc.declare_dram_parameter("w1", [EPG, P, KO, HB], f16, isOutput=False)
    w2_d = nc.declare_dram_parameter("w2", [EPG, P, HO, D_MODEL], f16, isOutput=False)
    b1_d = nc.declare_dram_parameter("b1", [P, EPG, HO], f32, isOutput=False)
    wb_d = nc.declare_dram_parameter("wb", [P, cols], f16, isOutput=False)
    # slab-blocked output: per slab a contiguous [P, DM*S] block at DM*col0
    out_d = nc.declare_dram_parameter("out", [P, DM * cols], f16, isOutput=True)

    seg_first_slab = {}
    for si, (sg, S, c0) in enumerate(slabs):
        seg_first_slab.setdefault(sg, si)
    segs_used = sorted(seg_first_slab)

    with TileContext(nc) as tc:
        with (
            tc.tile_pool(name="wpool", bufs=1) as wpool,
            tc.tile_pool(name="xpool", bufs=3) as xpool,
            tc.tile_pool(name="wbpool", bufs=3) as wbpool,
            tc.tile_pool(name="hpool", bufs=2) as hpool,
            tc.tile_pool(name="ypool", bufs=2) as ypool,
            tc.tile_pool(name="ps1", bufs=3, space="PSUM") as ps1,
            tc.tile_pool(name="ps2", bufs=4, space="PSUM") as ps2,
        ):
            # PE warm-up: dependency-free matmuls keep PE busy (and HAM
            # warming) through the preamble barrier + DMA lead-in.
            warm = wpool.tile([P, 512], f16)
            nc.vector.memset(warm, 0.0)
            wps = ps1.tile([P, SMAX], mybir.dt.float32, tag="hps")
            for _ in range(WARMUP_MM):
                nc.tensor.matmul(
                    wps[:, :512], lhsT=warm[:, :P], rhs=warm, start=True, stop=True
                )

            b1_sb = wpool.tile([P, EPG, HO], f32)
            w1_sb = wpool.tile([P, EPG, KO, HB], f16)
            w2_sb = wpool.tile([P, EPG, HO, D_MODEL], f16)

            # Weight stream in strict first-need order, ~1MB pieces so the
            # early HBM window (8 cores all loading) isn't oversubscribed.
            wq = []
            s0 = segs_used[0]
            for sg in segs_used:
                if sg == s0:
                    wq.append(("w1", sg, 0, 128))
                    wq.append(("w1", sg, 128, 512))
                    wq.append(("w1", sg, 512, HB))
                else:
                    wq.append(("w1", sg, 0, 512))
                    wq.append(("w1", sg, 512, HB))
                wq.append(("w2", sg, 0, 512))
                wq.append(("w2", sg, 512, D_MODEL))

            def issue_weight():
                if not wq:
                    return
                kind, sg, lo, hi = wq.pop(0)
                if kind == "w1":
                    nc.gpsimd.dma_start(
                        w1_sb[:, sg, :, lo:hi], w1_d[sg][:, :, lo:hi]
                    )
                else:
                    nc.gpsimd.dma_start(
                        w2_sb[:, sg, :, lo:hi], w2_d[sg][:, :, lo:hi]
                    )

            nc.scalar.dma_start(b1_sb, b1_d[:, :, :])
            # upfront: all of segment 0's weights (~4MB); the rest of the
            # stream is paced behind the per-slab out-DMAs on gpsimd's
            # in-order queue, so the 16MB never floods the early HBM window
            for _ in range(5):
                issue_weight()

            def mm1_slab(si):
                sg, S, c0 = slabs[si]
                x_sb = xpool.tile([P, KO, SMAX], f16, tag="x", name="x_sb")[:, :, :S]
                x_src = x_d[:, KO * c0 : KO * (c0 + S)].rearrange(
                    "p (ko t) -> p ko t", t=S
                )
                # two dma_starts -> two parallel HW queues
                nc.sync.dma_start(x_sb[:, : KO // 2, :], x_src[:, : KO // 2, :])
                nc.sync.dma_start(x_sb[:, KO // 2 :, :], x_src[:, KO // 2 :, :])
                wb_t = wbpool.tile([P, SMAX], f16, tag="wb", name="wb_t")[:, :S]
                nc.scalar.dma_start(wb_t, wb_d[:, c0 : c0 + S])
                h_sb = hpool.tile([P, HO, SMAX], f16, tag="h", name="h_sb")[:, :, :S]
                for ho in range(HO):
                    hps = ps1.tile(
                        [P, SMAX], mybir.dt.float32, tag="hps", name="hps"
                    )[:, :S]
                    for k in range(KO):
                        nc.tensor.matmul(
                            hps,
                            lhsT=w1_sb[:, sg, k, ho * P : (ho + 1) * P],
                            rhs=x_sb[:, k, :],
                            start=(k == 0),
                            stop=(k == KO - 1),
                        )
                    nc.scalar.activation(
                        h_sb[:, ho, :], hps, GELU, bias=b1_sb[:, sg, ho : ho + 1]
                    )
                    # fold the per-pair gate weight into H (fp16, free dim)
                    nc.vector.tensor_mul(h_sb[:, ho, :], h_sb[:, ho, :], wb_t)
                return h_sb

            def mm2_slab(si, h_sb):
                sg, S, c0 = slabs[si]
                last2 = si >= len(slabs) - 2
                y_all = ypool.tile([P, DM, SMAX], f16, tag="y", name="y_sb")[:, :, :S]
                out_dst = out_d[:, DM * c0 : DM * (c0 + S)].rearrange(
                    "p (m t) -> p m t", t=S
                )
                for m in range(DM):
                    yps = ps2.tile(
                        [P, SMAX], mybir.dt.float32, tag="yps", name="yps"
                    )[:, :S]
                    for ho in range(HO):
                        nc.tensor.matmul(
                            yps,
                            lhsT=w2_sb[:, sg, ho, m * P : (m + 1) * P],
                            rhs=h_sb[:, ho, :],
                            start=(ho == 0),
                            stop=(ho == HO - 1),
                        )
                    # PSUM->SBUF fp16 copy split across both engines
                    h2 = (S // 8) * 4
                    nc.scalar.copy(y_all[:, m, :h2], yps[:, :h2])
                    nc.vector.tensor_copy(y_all[:, m, h2:], yps[:, h2:])
                    # tail slabs: flush per-m / m-pairs from the (idle by
                    # then) sync engine so the out-DMA overlaps the matmuls
                    if si == len(slabs) - 1:
                        nc.sync.dma_start(out_dst[:, m : m + 1], y_all[:, m : m + 1])
                    elif last2 and m % 2 == 1:
                        nc.sync.dma_start(
                            out_dst[:, m - 1 : m + 1], y_all[:, m - 1 : m + 1]
                        )
                if not last2:
                    nc.gpsimd.dma_start(out_dst, y_all)
                    # pace the weight stream behind this slab's out-DMA
                    issue_weight()
                    issue_weight()

            # software pipeline: mm1(s+1) before mm2(s)
            h_prev = mm1_slab(0)
            for si in range(1, len(slabs)):
                h_cur = mm1_slab(si)
                mm2_slab(si - 1, h_prev)
                h_prev = h_cur
            mm2_slab(len(slabs) - 1, h_prev)
    nc.compile()
    return nc


def _route(moe_inp, Wg, bg):
    """Host gate: replicates NaiveGate (linear logits, top-2, softmax over the
    selected logits). Returns per-expert (token_idx, combine_weight)."""
    logits = moe_inp.astype(np.float32) @ Wg.astype(np.float32) + bg.astype(np.float32)
    order = np.argsort(-logits, axis=1, kind="stable")  # ties -> lower index first
    top_idx = order[:, :TOP_K]
    top_val = np.take_along_axis(logits, top_idx, axis=1)
    m = top_val.max(axis=1, keepdims=True)
    e = np.exp(top_val - m)
    gate = (e / e.sum(axis=1, keepdims=True)).astype(np.float32)
    toks, weights = [], []
    for ex in range(N_EXPERT):
        mask = top_idx == ex  # [N, K]; each token matches at most one slot
        t = np.nonzero(mask.any(axis=1))[0]
        w = gate[mask]  # row-major -> ascending token order, matches t
        toks.append(t)
        weights.append(w)
    return toks, weights


def kernel(**inputs):
    global last_results
    from concourse.bass_utils import run_bass_kernel_spmd

    moe_inp = np.asarray(inputs["moe_inp"], dtype=np.float32)
    Wg = np.asarray(inputs["Wg"], dtype=np.float32)
    bg = np.asarray(inputs["bg"], dtype=np.float32)
    W1 = np.asarray(inputs["W1"], dtype=np.float32)
    b1 = np.asarray(inputs["b1"], dtype=np.float32)
    W2 = np.asarray(inputs["W2"], dtype=np.float32)
    b2 = np.asarray(inputs["b2"], dtype=np.float32)

    toks, weights = _route(moe_inp, Wg, bg)
    loads = [len(t) for t in toks]
    groups, plan = _group_split(loads)
    slabs, cols = _make_slabs(plan)

    if slabs not in _nc_cache:
        _nc_cache[slabs] = _build_bass(slabs, cols)
    nc = _nc_cache[slabs]

    seg_c0 = {}
    for sg, S, c0 in slabs:
        if sg not in seg_c0:
            seg_c0[sg] = c0

    # per-group dispatched X^T / gate rows (segments padded to the plan)
    garrs = []
    for g in range(NGROUPS):
        xT = np.zeros((D_MODEL, cols), dtype=np.float16)
        wrow = np.zeros((cols,), dtype=np.float16)
        for i, e in enumerate(groups[g]):
            c0, L = seg_c0[i], loads[e]
            xT[:, c0 : c0 + L] = moe_inp[toks[e]].T
            wrow[c0 : c0 + L] = weights[e]
        blocks = []
        for sg, S, c0 in slabs:
            blocks.append(
                xT[:, c0 : c0 + S].reshape(KO, P, S).transpose(1, 0, 2).reshape(P, KO * S)
            )
        x_arr = np.ascontiguousarray(np.concatenate(blocks, axis=1))
        wb_arr = np.ascontiguousarray(np.broadcast_to(wrow, (P, cols)))
        garrs.append((x_arr, wb_arr))

    in_maps = []
    for c in range(NCORES):
        g, s = divmod(c, TPK)
        gex = list(groups[g])
        lo, hi = s * HB, (s + 1) * HB
        w1_arr = np.ascontiguousarray(
            W1[gex][:, :, lo:hi]
            .astype(np.float16)
            .reshape(EPG, KO, P, HB)
            .transpose(0, 2, 1, 3)
        )
        w2_arr = np.ascontiguousarray(
            W2[gex][:, lo:hi, :]
            .astype(np.float16)
            .reshape(EPG, HO, P, D_MODEL)
            .transpose(0, 2, 1, 3)
        )
        b1_arr = np.ascontiguousarray(
            b1[gex][:, lo:hi].reshape(EPG, HO, P).transpose(2, 0, 1)
        )
        in_maps.append(
            {
                "x": garrs[g][0],
                "w1": w1_arr,
                "w2": w2_arr,
                "b1": b1_arr,
                "wb": garrs[g][1],
            }
        )

    last_results = run_bass_kernel_spmd(nc, in_maps, core_ids=list(range(NCORES)))

    # host combine: per group sum the 4 hidden-slice partials, decode the
    # slab-blocked layout, scatter by segment
    out = np.zeros((N_TOKENS, D_MODEL), dtype=np.float32)
    for g in range(NGROUPS):
        raw = np.zeros((P, DM * cols), dtype=np.float32)
        for s in range(TPK):
            raw += last_results.results[g * TPK + s]["out"].astype(np.float32)
        yT = np.empty((D_MODEL, cols), dtype=np.float32)
        for sg, S, c0 in slabs:
            blk = raw[:, DM * c0 : DM * (c0 + S)].reshape(P, DM, S)
            yT[:, c0 : c0 + S] = blk.transpose(1, 0, 2).reshape(D_MODEL, S)
        for i, e in enumerate(groups[g]):
            c0, L = seg_c0[i], loads[e]
            out[toks[e]] += yT[:, c0 : c0 + L].T + weights[e][:, None] * b2[e][None, :]
    return out


if __name__ == "__main__":
    rng = np.random.default_rng(0)
    demo = {
        "moe_inp": rng.standard_normal((N_TOKENS, D_MODEL), dtype=np.float32),
        "attn_weights": rng.random((4, N_TOKENS, N_TOKENS), dtype=np.float32),
        "Wg": rng.standard_normal((D_MODEL, N_EXPERT), dtype=np.float32) / 32,
        "bg": np.zeros((N_EXPERT,), np.float32),
        "W1": rng.standard_normal((N_EXPERT, D_MODEL, D_HIDDEN), dtype=np.float32) / 32,
        "b1": np.zeros((N_EXPERT, D_HIDDEN), np.float32),
        "W2": rng.standard_normal((N_EXPERT, D_HIDDEN, D_MODEL), dtype=np.float32) / 64,
        "b2": np.zeros((N_EXPERT, D_MODEL), np.float32),
    }
    o = kernel(**demo)
    print(o.shape, o.dtype)
